# revision 24
# baseline (speedup 1.0000x reference)
"""Trainium2 Bass kernel for ChannelAttention1D.

Inputs (full): x (8, 256, 16384) f32, gamma (1,) f32.
  energy = einsum('bit,bjt->bij', x, x)
  att    = softmax(max_j(energy) - energy, axis=-1)
  out    = gamma * einsum('bij,bjt->bit', att, x) + x

Sharding: data-parallel over B across 8 NeuronCores (one batch per core).
The host passes x twice per core: exact f32 (for the +x epilogue) and a
bf16 copy (matmul operand dtype choice, prepared host-side in numpy).

Per-core kernel (C=256, T=16384):
  phase 1: DMA the bf16 copy in (resident, 8 MiB), PE-transpose 128x128
           blocks into xT tiles [128t, 256c], accumulate energy into PSUM
           (bf16 matmul, fp32 accumulate).  Energy is symmetric: only
           G00/G01 (rows 0:128 x all) and G11 are computed; G10 = G01.T is
           recovered with one extra PE transpose at softmax time.  The f32
           x stream for phase 2 prefetches through the same window.
  softmax: att = exp(rowmin - energy) / rowsum, algebraically identical to
           softmax(rowmax - energy).  gamma/rowsum is folded into the
           bf16 att operand (per-row scale before the transpose).
  phase 2: out = att_scaled @ x_bf16 + x_f32.  The +x runs in fp32 on DVE
           (scalar_tensor_tensor), so with gamma == 0 the kernel output is
           bit-exact x regardless of matmul precision.
"""

import os

import numpy as np
import ml_dtypes

import concourse.bacc as bacc
import concourse.bass as bass
import concourse.mybir as mybir
import concourse.tile as tile
from concourse.bass_utils import run_bass_kernel_spmd

F32 = mybir.dt.float32
BF16 = mybir.dt.bfloat16

B = 8
C = 256
T = 16384
N_CORES = 8
CH = 4096            # chunk width of the f32 stream
NCH = T // CH        # 4 chunks per 128-row block
XBCH = 4096          # chunk width of the resident bf16 copy
NXB = T // XBCH      # 4 bf16 chunks per 128-row block
NKT = T // 128       # 128 transpose+matmul steps for the energy accumulation
KB = 4               # phase-1 batch: 4 kt steps share one psum/sbuf tile
PO_N = 1024          # phase-2 psum tile width (2 fp32 PSUM banks)
XS_RES = 5           # f32 stream tiles resident during phase 1 (of 8)

LAST_RESULTS = None  # BassKernelResults of the most recent run (for test.py)


def _build_nc():
    nc = bacc.Bacc(
        "TRN2",
        target_bir_lowering=False,
        debug=False,
        enable_asserts=False,
        num_devices=N_CORES,
    )
    x_d = nc.dram_tensor("x", [C, T], F32, kind="ExternalInput")
    xb_d = nc.dram_tensor("xbf", [C, T], BF16, kind="ExternalInput")
    id_d = nc.dram_tensor("identity", [128, 128], BF16, kind="ExternalInput")
    g_d = nc.dram_tensor("gamma_b", [128, 1], F32, kind="ExternalInput")
    o_d = nc.dram_tensor("out", [C, T], F32, kind="ExternalOutput")

    Exp = mybir.ActivationFunctionType.Exp
    Copy = mybir.ActivationFunctionType.Copy
    Alu = mybir.AluOpType
    X = mybir.AxisListType.X

    with tile.TileContext(nc) as tc:
        with (
            tc.tile_pool(name="xbf", bufs=1) as xbpool,
            tc.tile_pool(name="xs", bufs=1) as xspool,
            tc.tile_pool(name="xt", bufs=4) as xtpool,
            tc.tile_pool(name="sm", bufs=1) as smpool,
            tc.tile_pool(name="outp", bufs=6) as outpool,
        ):
            # Resident bf16 chunks (first chunks DMA'd before anything else
            # so compute starts ASAP)
            xbf = [
                [
                    xbpool.tile([128, XBCH], BF16, tag=f"xb{m}_{c}", name=f"xb{m}_{c}")
                    for c in range(NXB)
                ]
                for m in range(2)
            ]
            # identity first (every transpose streams it), then the first
            # chunk in quarters, alternating row blocks, so the very first
            # transposes have data as early as possible
            ident = smpool.tile([128, 128], BF16, tag="ident", name="ident")
            nc.sync.dma_start(ident[:], id_d.ap())
            H = XBCH // 2
            for h in range(2):
                for m in range(2):
                    nc.sync.dma_start(
                        xbf[m][0][:, h * H:(h + 1) * H],
                        xb_d.ap()[m * 128:(m + 1) * 128, h * H:(h + 1) * H],
                    )
            g128 = smpool.tile([128, 1], F32, tag="g128", name="g128")
            nc.scalar.dma_start(g128[:], g_d.ap())

            # f32 stream for the phase-2 epilogue: XS_RES tiles prefetch
            # during phase 1; the rest reuse the earliest-consumed slots.
            xs_tiles = [
                xspool.tile([128, CH], F32, tag=f"xs{i % XS_RES}", name=f"xs{i}")
                for i in range(2 * NCH)
            ]

            def xs_dma(idx):
                m, c = divmod(idx, NCH)
                nc.scalar.dma_start(
                    xs_tiles[idx][:],
                    x_d.ap()[m * 128:(m + 1) * 128, c * CH:(c + 1) * CH],
                )

            e_bf, gsc, eT = [], [], []

            with (
                tc.tile_pool(name="pt", bufs=3, space=bass.MemorySpace.PSUM) as ptpool,
                tc.tile_pool(name="pe", bufs=1, space=bass.MemorySpace.PSUM) as pepool,
            ):
                # Energy accumulators (PSUM-resident for all of phase 1).
                # Rows 0:128 need all 256 cols; rows 128:256 only cols
                # 128:256 (G10 = G01.T by symmetry).
                pe0 = pepool.tile([128, C], F32, tag="pe0", name="pe0")
                pe1 = pepool.tile([128, 128], F32, tag="pe1", name="pe1")

                def energy_mms(xt_big, k0):
                    """xt_big: [128, KB*C] bf16 holding KB consecutive xT
                    tiles; emit 2*KB accumulation matmuls."""
                    for j in range(KB):
                        k = k0 + j
                        xt = xt_big[:, j * C:(j + 1) * C]
                        nc.tensor.matmul(
                            pe0[:], xt[:, 0:128], xt[:],
                            start=(k == 0), stop=(k == NKT - 1),
                        )
                        nc.tensor.matmul(
                            pe1[:], xt[:, 128:256], xt[:, 128:256],
                            start=(k == 0), stop=(k == NKT - 1),
                        )

                # ---- phase 1: transpose + energy accumulation ----
                pending = []  # [(xt_big, k0), ...] 2-batch skew so the PE
                # matmuls never stall on the DVE psum->sbuf copy
                k = 0
                for c in range(NXB):
                    if c > 0:
                        for h2 in range(2):
                            for m in range(2):
                                lo = c * XBCH + h2 * H
                                nc.sync.dma_start(
                                    xbf[m][c][:, h2 * H:(h2 + 1) * H],
                                    xb_d.ap()[m * 128:(m + 1) * 128, lo:lo + H],
                                )
                        if c >= 2:
                            for i in range((c - 2) * 4, min((c - 1) * 4, XS_RES)):
                                xs_dma(i)
                    for sb in range(XBCH // (128 * KB)):
                        pt = ptpool.tile([128, KB * C], BF16, tag="pt", name="pt")
                        for j in range(KB):
                            s = sb * KB + j
                            for m in range(2):
                                nc.tensor.transpose(
                                    pt[:, j * C + m * 128:j * C + (m + 1) * 128],
                                    xbf[m][c][:, s * 128:(s + 1) * 128],
                                    ident[:],
                                )
                        xt_big = xtpool.tile(
                            [128, KB * C], BF16, tag="xt", name="xt"
                        )
                        nc.vector.tensor_copy(xt_big[:], pt[:])
                        pending.append((xt_big, k))
                        if len(pending) > 2:
                            energy_mms(*pending.pop(0))
                        k += KB
                for p in pending:
                    energy_mms(*p)

                # ---- softmax epilogue ----
                # G10 = G01.T: copy psum G01 -> sbuf (bf16), PE-transpose.
                g01b = smpool.tile([128, 128], BF16, tag="g01", name="g01")
                nc.vector.tensor_copy(g01b[:], pe0[:, 128:256])
                ptg = ptpool.tile([128, KB * C], BF16, tag="pt", name="pt")
                nc.tensor.transpose(ptg[:, 0:128], g01b[:], ident[:])

                for m in range(2):
                    e = smpool.tile([128, C], F32, tag=f"e{m}", name=f"e{m}")
                    rsum = smpool.tile([128, 1], F32, tag=f"rs{m}", name=f"rs{m}")
                    rmin = smpool.tile([128, 1], F32, tag=f"rm{m}", name=f"rm{m}")
                    if m == 0:
                        nc.vector.tensor_reduce(
                            rmin[:], pe0[:], axis=X, op=Alu.min
                        )
                        nc.scalar.activation(
                            e[:], pe0[:], Exp, bias=rmin[:], scale=-1.0,
                            accum_out=rsum[:],
                        )
                    else:
                        ra = smpool.tile([128, 1], F32, tag="ra", name="ra")
                        rb = smpool.tile([128, 1], F32, tag="rb", name="rb")
                        nc.vector.tensor_reduce(
                            ra[:], ptg[:, 0:128], axis=X, op=Alu.min
                        )
                        nc.vector.tensor_reduce(
                            rb[:], pe1[:], axis=X, op=Alu.min
                        )
                        nc.vector.scalar_tensor_tensor(
                            rmin[:], ra[:], 0.0, rb[:],
                            op0=Alu.bypass, op1=Alu.min,
                        )
                        sa = smpool.tile([128, 1], F32, tag="sa", name="sa")
                        sb_ = smpool.tile([128, 1], F32, tag="sb", name="sb")
                        nc.scalar.activation(
                            e[:, 0:128], ptg[:, 0:128], Exp, bias=rmin[:],
                            scale=-1.0, accum_out=sa[:],
                        )
                        nc.scalar.activation(
                            e[:, 128:256], pe1[:], Exp, bias=rmin[:],
                            scale=-1.0, accum_out=sb_[:],
                        )
                        nc.vector.scalar_tensor_tensor(
                            rsum[:], sa[:], 0.0, sb_[:],
                            op0=Alu.bypass, op1=Alu.add,
                        )
                    rinv = smpool.tile([128, 1], F32, tag=f"ri{m}", name=f"ri{m}")
                    nc.vector.reciprocal(rinv[:], rsum[:])
                    g = smpool.tile([128, 1], F32, tag=f"gs{m}", name=f"gs{m}")
                    nc.vector.scalar_tensor_tensor(
                        g[:], rinv[:], 0.0, g128[:], op0=Alu.bypass, op1=Alu.mult
                    )
                    # fold gamma/rowsum into the bf16 att operand (per-row)
                    eb = smpool.tile([128, C], BF16, tag=f"eb{m}", name=f"eb{m}")
                    nc.scalar.activation(eb[:], e[:], Copy, scale=g[:])
                    e_bf.append(eb)
                    gsc.append(g)

                # eT[kc][j, i] = att_scaled[i, kc*128 + j]
                for kc in range(2):
                    pt2 = ptpool.tile([128, KB * C], BF16, tag="pt", name="pt")
                    for mi in range(2):
                        nc.tensor.transpose(
                            pt2[:, mi * 128:(mi + 1) * 128],
                            e_bf[mi][:, kc * 128:(kc + 1) * 128],
                            ident[:],
                        )
                    t = smpool.tile([128, C], BF16, tag=f"eT{kc}", name=f"eT{kc}")
                    nc.vector.tensor_copy(t[:], pt2[:, 0:C])
                    eT.append(t)

            # ---- phase 2: out = att_scaled @ x_bf16 + x_f32 ----
            with tc.tile_pool(
                name="po", bufs=4, space=bass.MemorySpace.PSUM
            ) as popool:
                for m in range(2):
                    for c in range(NCH):
                        idx = m * NCH + c
                        xs = xs_tiles[idx]
                        for h in range(CH // PO_N):
                            po = popool.tile([128, PO_N], F32, tag="po", name="po")
                            for q in range(PO_N // 512):
                                col = h * PO_N + q * 512
                                gcol = c * CH + col
                                xc, xo = divmod(gcol, XBCH)
                                for kc in range(2):
                                    nc.tensor.matmul(
                                        po[:, q * 512:(q + 1) * 512],
                                        eT[kc][:, m * 128:(m + 1) * 128],
                                        xbf[kc][xc][:, xo:xo + 512],
                                        start=(kc == 0),
                                        stop=(kc == 1),
                                    )
                            outc = outpool.tile(
                                [128, PO_N], F32, tag="outc", name="outc"
                            )
                            nc.vector.scalar_tensor_tensor(
                                outc[:],
                                po[:],
                                0.0,
                                xs[:, h * PO_N:(h + 1) * PO_N],
                                op0=Alu.bypass,
                                op1=Alu.add,
                            )
                            nc.sync.dma_start(
                                o_d.ap()[
                                    m * 128:(m + 1) * 128,
                                    c * CH + h * PO_N:c * CH + (h + 1) * PO_N,
                                ],
                                outc[:],
                            )
                        # refill the f32 stream window
                        if idx + XS_RES < 2 * NCH:
                            xs_dma(idx + XS_RES)

    nc.compile()
    return nc


_NC_CACHE = None


def _get_nc():
    global _NC_CACHE
    if _NC_CACHE is None:
        _NC_CACHE = _build_nc()
    return _NC_CACHE


def kernel(x, gamma):
    x = np.ascontiguousarray(np.asarray(x, dtype=np.float32))
    g = np.asarray(gamma, dtype=np.float32).reshape(-1)
    assert x.shape == (B, C, T), x.shape

    nc = _get_nc()
    xbf = x.astype(ml_dtypes.bfloat16)
    ident = np.eye(128, dtype=ml_dtypes.bfloat16)
    gb = np.full((128, 1), g[0], dtype=np.float32)
    in_maps = [
        {
            "x": np.ascontiguousarray(x[b]),
            "xbf": np.ascontiguousarray(xbf[b]),
            "identity": ident,
            "gamma_b": gb,
        }
        for b in range(B)
    ]

    trace = os.environ.get("KERNEL_TRACE", "0") == "1"
    res = run_bass_kernel_spmd(
        nc, in_maps, core_ids=list(range(N_CORES)), trace=trace
    )
    global LAST_RESULTS
    LAST_RESULTS = res
    return np.stack([r["out"] for r in res.results], axis=0)
